# revision 1
# baseline (speedup 1.0000x reference)
"""Trainium2 Bass kernel for nn_Aggregator (segment_reduce):
res[b,d] = sum_n mask[b,n] * (nodes@Wt.T + bt)[n,d] * sigmoid(nodes@Wg.T + bg)[n,d]

Sharding: nodes and owner_masks split along N across 8 NeuronCores; params
replicated; per-core partial [B,D] summed on host.

Host-side prep (part of sharding): nodes are transposed to [D_IN, N] and
owner_masks to [N, B], both regrouped per 3584-node DMA group so every SBUF
partition reads one contiguous run, and cast to the compute dtype. This puts
the contraction dim on partitions for every matmul, so the device does no
transposes at all:

  per 128-node subchunk s (contraction dim on partitions throughout):
    dg[n, 0:512] = nodesT[:, s].T @ [WtT | WgT]     (2 accumulating matmuls,
                                                     one PSUM bank, 4 bufs)
    gates = sigmoid(dg[:, 256:512] + bg)            (ACT; bg fused as scalar
                                                     activation bias)
    prod  = (dg[:, 0:256] + bt) * gates             (DVE; bf16 product)
    res[b, :] += maskT[:, s].T @ prod               (2 accumulating matmuls
                                                     into persistent PSUM)

  Warmup matmuls on a zeroed tile run during the initial DMA fill so the
  PE's HAM clock gate is already at 2.4 GHz when real work arrives. Bias
  constants stay fp32: quantizing them would add a per-column offset that
  accumulates coherently over the 25k-node reduction.

Modes ("mix" default, select with BASS_AGG_MODE):
  mix:  data/mask matmuls in bf16; the gates matmul runs as a single fp8e4m3
        DoubleRow matmul (2 packed weights/cell, both 128-feature chunks in
        one 0.5-cyc/row pass; sigmoid compresses the fp8 error 4x). Aux ops
        batched per subchunk pair. ~117us est., rel err ~6e-4.
  bf16: all matmul operands bf16, fp32 PSUM accum. ~136us, rel err ~6e-5.
  f32r: float32r (tf32-like) storage, ~169us, rel err ~4e-6.
"""

import os
import sys
from contextlib import ExitStack

import numpy as np

sys.path.insert(0, "/opt/trn_rl_repo")

import concourse.bass as bass  # noqa: E402
import concourse.tile as tile  # noqa: E402
from concourse import bacc, mybir  # noqa: E402
from concourse.bass_utils import run_bass_kernel_spmd  # noqa: E402

N, D_IN, D_OUT, B = 200000, 256, 256, 256
NCORES = 8
CHUNK = 128          # nodes per subchunk (one matmul block)
GROUP = 3584         # nodes per DMA group
NSH = 25088          # padded nodes per core (= 196 * 128 = 7 * 3584)
NGROUPS = NSH // GROUP       # 7
SUBS = GROUP // CHUNK        # 28 subchunks per group (even -> 14 pairs)

F32 = mybir.dt.float32
F32R = mybir.dt.float32r
BF16 = mybir.dt.bfloat16
FP8 = mybir.dt.float8e4

MODE = os.environ.get("BASS_AGG_MODE", "mix")

_BUILT = {}
_LAST_BG_SCALAR = 1.0


def _build(mode, bg_scalar):
    cdt = F32R if mode == "f32r" else BF16
    nc = bacc.Bacc("TRN2", target_bir_lowering=False, debug=False,
                   num_devices=NCORES)

    # nodesT grouped: [g][p][k*GROUP + n] = nodesT[k*128+p, g*GROUP+n]
    ndT = nc.dram_tensor("ndT", [NGROUPS, 128, 2 * GROUP], cdt,
                         kind="ExternalInput").ap()
    # maskT grouped: [g][p][s*256 + b] = maskT[g*GROUP + s*128 + p, b]
    mkT = nc.dram_tensor("mkT", [NGROUPS, 128, SUBS * 256], cdt,
                         kind="ExternalInput").ap()
    mix = (mode == "mix")
    WFW = 512 if mix else 4 * D_OUT
    wf = nc.dram_tensor("wf", [128, WFW], cdt, kind="ExternalInput").ap()
    if mix:
        # fp8 copies for the DoubleRow gates matmul
        ndf = nc.dram_tensor("ndf", [NGROUPS, 128, 2 * GROUP], FP8,
                             kind="ExternalInput").ap()
        wgf = nc.dram_tensor("wgf", [128, 512], FP8,
                             kind="ExternalInput").ap()
    if bg_scalar is None:
        bfull = nc.dram_tensor("bfull", [128, 1024], F32,
                               kind="ExternalInput").ap()
    else:
        btile = nc.dram_tensor("btile", [128, 512], F32,
                               kind="ExternalInput").ap()
    out_res = nc.dram_tensor("res", [B, D_OUT], F32, kind="ExternalOutput").ap()

    SIG = mybir.ActivationFunctionType.Sigmoid

    with tile.TileContext(nc) as tc, ExitStack() as ctx:
        const = ctx.enter_context(tc.tile_pool(name="const", bufs=1))
        gio = ctx.enter_context(tc.tile_pool(name="gio", bufs=2))
        work = ctx.enter_context(tc.tile_pool(name="work", bufs=3))
        ppsb = 3 if (mix and bg_scalar is not None) else 4
        pps = ctx.enter_context(tc.tile_pool(name="pps", bufs=ppsb,
                                             space="PSUM"))
        rps = ctx.enter_context(tc.tile_pool(name="rps", bufs=1, space="PSUM"))

        # the first matmul's WEIGHTS are the first 128 nodes: land their
        # slice before anything else occupies the DMA queue head
        NSP0 = 8
        g0_nd = gio.tile([128, 2 * GROUP], cdt, tag="nd")
        g0_nd3d = g0_nd[:].rearrange("p (k n) -> p k n", k=2)
        nc.sync.dma_start(g0_nd3d[:, :, 0:GROUP // NSP0],
                          ndT[0].rearrange("p (k n) -> p k n", k=2)
                          [:, :, 0:GROUP // NSP0])

        wf_s = const.tile([128, WFW], cdt)
        nc.scalar.dma_start(wf_s[:], wf[:])
        if mix:
            wgf_s = const.tile([128, 512], FP8)
            nc.scalar.dma_start(wgf_s[:], wgf[:])
            wgf3 = wgf_s[:].rearrange("p (k d) -> p k d", k=2)
        if bg_scalar is None:
            bf_s = const.tile([128, 1024], F32)
            nc.scalar.dma_start(bf_s[:], bfull[:])
        else:
            bt_s = const.tile([128, 512], F32)
            nc.scalar.dma_start(bt_s[:], btile[:])

        res0 = rps.tile([128, D_OUT], F32)
        res1 = rps.tile([128, D_OUT], F32)

        # warm up the PE (HAM clock gate) with junk matmuls on a zeroed tile
        # while the first DMAs are still in flight; without this the first
        # ~3us of real matmuls run at the cold 1.2 GHz clock
        wz = const.tile([128, 64], BF16)
        nc.vector.memset(wz[:], 0.0)
        # warmups write junk into res0; the real accumulation's start=True
        # overwrites it, so no dedicated psum bank is needed
        for _ in range(64):
            nc.tensor.matmul(res0[0:64, 0:64], wz[:], wz[:],
                             start=True, stop=True)

        for g in range(NGROUPS):
            # split each group load into n-slices: finer completion
            # granularity -> compute starts sooner, fewer mid-loop stalls.
            # group 0 is split finer so the pipeline fills fast.
            nsp = NSP0 if g == 0 else 4
            nd_s = g0_nd if g == 0 else gio.tile([128, 2 * GROUP], cdt,
                                                 tag="nd")
            mk_s = gio.tile([128, SUBS * 256], cdt, tag="mk")
            nd3d = nd_s[:].rearrange("p (k n) -> p k n", k=2)
            ndg = ndT[g].rearrange("p (k n) -> p k n", k=2)
            if mix:
                ndf_s = gio.tile([128, 2 * GROUP], FP8, tag="ndf")
                ndf3 = ndf_s[:].rearrange("p (k n) -> p k n", k=2)
                ndfg = ndf[g].rearrange("p (k n) -> p k n", k=2)
            W = SUBS * 256
            for q in range(nsp):
                lo, hi = q * GROUP // nsp, (q + 1) * GROUP // nsp
                if not (g == 0 and q == 0):
                    nc.sync.dma_start(nd3d[:, :, lo:hi], ndg[:, :, lo:hi])
                if mix:
                    nc.sync.dma_start(ndf3[:, :, lo:hi], ndfg[:, :, lo:hi])
                lo, hi = q * W // nsp, (q + 1) * W // nsp
                nc.sync.dma_start(mk_s[:, lo:hi], mkT[g][:, lo:hi])

            for p in range(SUBS // 2):
                s0 = 2 * p
                first = (g == 0 and p == 0)
                last = (g == NGROUPS - 1 and p == SUBS // 2 - 1)

                adt = F32 if mode == "f32r" else BF16
                gt_s = work.tile([128, 512], adt, tag="gts")
                db_s = work.tile([128, 512], adt, tag="dbs")
                if bg_scalar is None:
                    dgb = work.tile([128, 1024], adt, tag="dgb")
                else:
                    dgb = None
                dg_pair = [None, None]
                if mix and bg_scalar is not None:
                    # per-pair 2-bank psum + batched aux ops: PE has slack
                    # in mix mode, and batching halves DVE op overheads
                    dgp2 = pps.tile([128, 1024], F32, tag="dgp2")
                # per-subchunk single-bank psum tiles -> deeper PE pipelining
                for k in range(2):
                    s = s0 + k
                    if mix and bg_scalar is not None:
                        dg_ps = dgp2[:, k * 512:(k + 1) * 512]
                    else:
                        dg_ps = pps.tile([128, 512], F32, tag="dgp")
                    dg_pair[k] = dg_ps
                    if mix:
                        # data: two bf16 matmuls; gates: one fp8 DoubleRow
                        # matmul contracting both 128-feature chunks
                        nc.tensor.matmul(dg_ps[:, 0:256],
                                         nd_s[:, s * 128:(s + 1) * 128],
                                         wf_s[:, 0:256],
                                         start=True, stop=False)
                        nc.tensor.matmul(dg_ps[:, 0:256],
                                         nd_s[:, GROUP + s * 128:
                                               GROUP + (s + 1) * 128],
                                         wf_s[:, 256:512],
                                         start=False, stop=True)
                        nc.tensor.matmul(
                            dg_ps[:, 256:512],
                            ndf3[:, :, s * 128:(s + 1) * 128], wgf3,
                            start=True, stop=True,
                            perf_mode=mybir.MatmulPerfMode.DoubleRow)
                    else:
                        nc.tensor.matmul(dg_ps[:],
                                         nd_s[:, s * 128:(s + 1) * 128],
                                         wf_s[:, 0:512],
                                         start=True, stop=False)
                        nc.tensor.matmul(dg_ps[:],
                                         nd_s[:, GROUP + s * 128:
                                               GROUP + (s + 1) * 128],
                                         wf_s[:, 512:1024],
                                         start=False, stop=True)
                    o = k * 256
                    if bg_scalar is None:
                        nc.vector.tensor_add(dgb[:, 2 * o:2 * o + 512],
                                             dg_ps[:], bf_s[:, 0:512])
                        nc.scalar.activation(gt_s[:, o:o + 256],
                                             dgb[:, 2 * o + 256:2 * o + 512],
                                             SIG)
                    elif not mix:
                        nc.scalar.activation(gt_s[:, o:o + 256],
                                             dg_ps[:, 256:512], SIG,
                                             bias=float(bg_scalar), scale=1.0)
                        nc.vector.tensor_add(db_s[:, o:o + 256],
                                             dg_ps[:, 0:256], bt_s[:, 0:256])

                pr_s = work.tile([128, 512], cdt, tag="prs")
                if mix and bg_scalar is not None:
                    dg4 = dgp2[:].rearrange("q (s h d) -> q s h d",
                                            s=2, d=256)
                    bt3 = bt_s[:].rearrange("q (s d) -> q s d", s=2)
                    db3 = db_s[:].rearrange("q (s d) -> q s d", s=2)
                    gt3 = gt_s[:].rearrange("q (s d) -> q s d", s=2)
                    nc.scalar.activation(gt3, dg4[:, :, 1, :], SIG,
                                         bias=float(bg_scalar), scale=1.0)
                    nc.vector.tensor_add(db3, dg4[:, :, 0, :], bt3)
                    nc.vector.tensor_mul(pr_s[:], db_s[:], gt_s[:])
                else:
                    for k in range(2):
                        o = k * 256
                        if bg_scalar is None:
                            nc.vector.tensor_mul(pr_s[:, o:o + 256],
                                                 dgb[:, 2 * o:2 * o + 256],
                                                 gt_s[:, o:o + 256])
                        else:
                            nc.vector.tensor_mul(pr_s[:, o:o + 256],
                                                 db_s[:, o:o + 256],
                                                 gt_s[:, o:o + 256])

                for k in range(2):
                    s = s0 + k
                    kfirst = first and k == 0
                    klast = last and k == 1
                    nc.tensor.matmul(res0[:], mk_s[:, s * 256:s * 256 + 128],
                                     pr_s[:, k * 256:(k + 1) * 256],
                                     start=kfirst, stop=klast)
                    nc.tensor.matmul(res1[:],
                                     mk_s[:, s * 256 + 128:s * 256 + 256],
                                     pr_s[:, k * 256:(k + 1) * 256],
                                     start=kfirst, stop=klast)

        rs = work.tile([128, 2 * D_OUT], F32, tag="rout")
        nc.vector.tensor_copy(rs[:, 0:256], res0[:])
        nc.scalar.copy(rs[:, 256:512], res1[:])
        out3d = out_res.rearrange("(h b) d -> b h d", h=2)
        nc.sync.dma_start(out3d, rs[:].rearrange("b (h d) -> b h d", h=2))

    nc.compile()
    return nc


def _get_nc(bg_scalar, mode=None):
    mode = mode or MODE
    key = (mode, None if bg_scalar is None else float(bg_scalar))
    if key not in _BUILT:
        _BUILT[key] = _build(mode, bg_scalar)
    return _BUILT[key]


def _prep_host(nodes, owner_masks, np_cdt):
    """Pad, shard, transpose + regroup nodes/masks into the DMA layouts.
    Cast to the compute dtype first so the big strided copies move half
    the bytes."""
    ntot = NCORES * NSH
    nd = np.zeros((ntot, D_IN), np_cdt)
    nd[:N] = nodes                       # cast f32 -> cdt
    # ndT[c, g, p, k, n] = nodes[c*NSH + g*GROUP + n, k*128 + p]
    ndr = nd.reshape(NCORES, NGROUPS, GROUP, 2, 128)
    ndT = np.ascontiguousarray(ndr.transpose(0, 1, 4, 3, 2)).reshape(
        NCORES, NGROUPS, 128, 2 * GROUP)

    mk = np.zeros((B, ntot), np_cdt)
    mk[:, :N] = owner_masks              # cast int -> cdt (0/1 exact)
    # mkT[c, g, p, s, b] = mask[b, c*NSH + g*GROUP + s*128 + p]
    mkr = mk.reshape(B, NCORES, NGROUPS, SUBS, 128)
    mkT = np.ascontiguousarray(mkr.transpose(1, 2, 4, 3, 0)).reshape(
        NCORES, NGROUPS, 128, SUBS * B)

    return [(ndT[c], mkT[c]) for c in range(NCORES)]


def kernel(nodes, owner_masks, Wt, bt, Wg, bg, _spmd_extra_kwargs=None):
    import ml_dtypes

    nodes = np.asarray(nodes, dtype=np.float32)
    owner_masks = np.asarray(owner_masks)
    Wt = np.asarray(Wt, dtype=np.float32)
    bt = np.asarray(bt, dtype=np.float32)
    Wg = np.asarray(Wg, dtype=np.float32)
    bg = np.asarray(bg, dtype=np.float32)

    bg_scalar = float(bg[0]) if np.all(bg == bg[0]) else None
    mix = (MODE == "mix")
    global _LAST_BG_SCALAR
    _LAST_BG_SCALAR = bg_scalar
    nc = _get_nc(bg_scalar)
    np_cdt = np.float32 if MODE == "f32r" else ml_dtypes.bfloat16

    shards = _prep_host(nodes, owner_masks, np_cdt)

    WtT, WgT = Wt.T, Wg.T
    if mix:
        # wf: [WtT chunk0 | WtT chunk1]; gates weights go in fp8 pairs
        wf_np = np.empty((128, 512), np.float32)
        wf_np[:, 0:256] = WtT[0:128]
        wf_np[:, 256:512] = WtT[128:256]
    else:
        # wf: rows = i-chunk features, cols = [WtT | WgT] per chunk
        wf_np = np.empty((128, 4 * D_OUT), np.float32)
        wf_np[:, 0:256] = WtT[0:128]
        wf_np[:, 256:512] = WgT[0:128]
        wf_np[:, 512:768] = WtT[128:256]
        wf_np[:, 768:1024] = WgT[128:256]
    if mix:
        import ml_dtypes as _mld
        wgf_np = np.empty((128, 512), np.float32)
        wgf_np[:, 0:256] = WgT[0:128]
        wgf_np[:, 256:512] = WgT[128:256]
        mix_common = {"wgf": wgf_np.astype(_mld.float8_e4m3)}
    if bg_scalar is None:
        common = {"wf": wf_np.astype(np_cdt)}
        bfull = np.empty((128, 1024), np.float32)
        for k in range(2):
            bfull[:, k * 512:k * 512 + 256] = bt
            bfull[:, k * 512 + 256:(k + 1) * 512] = bg
        common["bfull"] = bfull
    else:
        bt2 = np.empty((128, 512), np.float32)
        bt2[:, 0:256] = bt
        bt2[:, 256:512] = bt
        common = {"wf": wf_np.astype(np_cdt), "btile": bt2}

    if mix:
        common.update(mix_common)
        in_maps = [{"ndT": ndTg, "mkT": mkTg,
                    "ndf": ndTg.astype(ml_dtypes.float8_e4m3), **common}
                   for (ndTg, mkTg) in shards]
    else:
        in_maps = [{"ndT": ndTg, "mkT": mkTg, **common}
                   for (ndTg, mkTg) in shards]

    extra = _spmd_extra_kwargs or {}
    res = run_bass_kernel_spmd(nc, in_maps, list(range(NCORES)), **extra)
    out = np.zeros((B, D_OUT), np.float64)
    for c in range(NCORES):
        out += res.results[c]["res"].astype(np.float64)
    kernel.last_results = res
    return out.astype(np.float32)

